# revision 4
# baseline (speedup 1.0000x reference)
import sys

for _p in (
    "/root/.axon_site",
    "/root/.axon_site/_ro/trn_rl_repo",
    "/root/.axon_site/_ro/pypackages",
    "/opt/trn_rl_repo",
):
    if _p not in sys.path:
        sys.path.append(_p)

import numpy as np

# Problem constants (nn_AppearanceComposability: B,C,H,W = 4,64,256,256, k=3, stride=1)
B, C, H, W = 4, 64, 256, 256
K = 3
T = K * K           # 9 taps
WO = W - K + 1      # 254 output cols
HO = H - K + 1      # 254 output rows
NPLANES = B * C     # 256 (b,c) planes
NCORES = 8
ROWS = 32           # output rows per core (8*32 = 256 >= 254, last core padded)
R = 4               # output rows per compute block
KR = ROWS + K - 1   # key rows needed per core = 34

# taps (kh*3+kw) that run on GpSimd instead of DVE (both run tensor_tensor
# at ~equal rate; 4/9 on GpSimd balances the two engines)
GPSIMD_TAPS = (3, 4, 5, 6)

_CACHE = {}


def _build_nc():
    import concourse.bass as bass
    import concourse.mybir as mybir
    from concourse import bacc
    from concourse.tile import TileContext

    f32 = mybir.dt.float32
    nc = bacc.Bacc("TRN2", target_bir_lowering=False, debug=False, num_devices=NCORES)
    key = nc.declare_dram_parameter("key", [NPLANES, KR * W], f32, isOutput=False)
    query = nc.declare_dram_parameter("query", [NPLANES, ROWS * W], f32, isOutput=False)
    out = nc.declare_dram_parameter("out", [NPLANES, ROWS * WO * T], f32, isOutput=True)

    with TileContext(nc) as tc:
        with (
            tc.tile_pool(name="kq", bufs=3) as kq_pool,
            tc.tile_pool(name="op", bufs=4) as out_pool,
        ):
            for g in range(NPLANES // 128):
                for blk in range(ROWS // R):
                    r0 = blk * R
                    ktile = kq_pool.tile([128, (R + 2) * W], f32, tag="key")
                    nc.scalar.dma_start(
                        out=ktile[:],
                        in_=key[g * 128:(g + 1) * 128, r0 * W:(r0 + R + 2) * W],
                    )
                    qtile = kq_pool.tile([128, R * W], f32, tag="query")
                    nc.scalar.dma_start(
                        out=qtile[:],
                        in_=query[g * 128:(g + 1) * 128, r0 * W:(r0 + R) * W],
                    )
                    otile = out_pool.tile([128, R * WO * T], f32, tag="out")
                    kv = ktile[:].rearrange("p (r w) -> p r w", w=W)
                    qv = qtile[:].rearrange("p (r w) -> p r w", w=W)
                    ov = otile[:].rearrange("p (r w t) -> p r w t", w=WO, t=T)
                    for kh in range(K):
                        for kw in range(K):
                            t = kh * K + kw
                            eng = nc.gpsimd if t in GPSIMD_TAPS else nc.vector
                            eng.tensor_mul(
                                ov[:, :, :, t],
                                kv[:, kh:kh + R, kw:kw + WO],
                                qv[:, :, 1:1 + WO],
                            )
                    nc.sync.dma_start(
                        out=out[g * 128:(g + 1) * 128, r0 * WO * T:(r0 + R) * WO * T],
                        in_=otile[:],
                    )
    nc.compile()
    return nc


def _get_nc():
    if "nc" not in _CACHE:
        _CACHE["nc"] = _build_nc()
    return _CACHE["nc"]


def _make_in_maps(key_map, query_map):
    kflat = np.ascontiguousarray(key_map.reshape(NPLANES, H, W))
    qflat = np.ascontiguousarray(query_map.reshape(NPLANES, H, W))
    in_maps = []
    for i in range(NCORES):
        r0 = ROWS * i
        kshard = np.zeros((NPLANES, KR, W), np.float32)
        nrows = min(KR, H - r0)
        kshard[:, :nrows] = kflat[:, r0:r0 + nrows]
        qshard = np.zeros((NPLANES, ROWS, W), np.float32)
        qrows = min(ROWS, H - (r0 + 1))
        qshard[:, :qrows] = qflat[:, r0 + 1:r0 + 1 + qrows]
        in_maps.append({
            "key": kshard.reshape(NPLANES, KR * W),
            "query": qshard.reshape(NPLANES, ROWS * W),
        })
    return in_maps


def run_spmd(key_map, query_map, trace=False, **kwargs):
    from concourse.bass_utils import run_bass_kernel_spmd

    nc = _get_nc()
    in_maps = _make_in_maps(key_map, query_map)
    res = run_bass_kernel_spmd(
        nc, in_maps, core_ids=list(range(NCORES)), trace=trace, **kwargs
    )
    outs = [res.results[i]["out"].reshape(NPLANES, ROWS, WO, K, K)
            for i in range(NCORES)]
    full = np.concatenate(outs, axis=1)[:, :HO]
    return full.reshape(B, C, HO * WO, K, K), res


def kernel(key_map, query_map, k, stride):
    assert int(k) == K and int(stride) == 1
    key_map = np.asarray(key_map, dtype=np.float32)
    query_map = np.asarray(query_map, dtype=np.float32)
    out, _ = run_spmd(key_map, query_map, trace=False)
    return out


# revision 6
# speedup vs baseline: 1.0832x; 1.0832x over previous
import sys

for _p in (
    "/root/.axon_site",
    "/root/.axon_site/_ro/trn_rl_repo",
    "/root/.axon_site/_ro/pypackages",
    "/opt/trn_rl_repo",
):
    if _p not in sys.path:
        sys.path.append(_p)

import numpy as np

# Problem constants (nn_AppearanceComposability: B,C,H,W = 4,64,256,256, k=3, stride=1)
B, C, H, W = 4, 64, 256, 256
K = 3
T = K * K           # 9 taps
WO = W - K + 1      # 254 output cols
HO = H - K + 1      # 254 output rows
NPLANES = B * C     # 256 (b,c) planes
NCORES = 8
ROWS = 32           # output rows per core (8*32 = 256 >= 254, last core padded)
R = 4               # output rows per compute block
KR = ROWS + K - 1   # key rows needed per core = 34

# DVE and GpSimd contend for SBUF ports (concurrent elementwise runs both at
# ~half rate), so all tap multiplies stay on DVE using wide [R, WO, K] APs
# with a stride-0 broadcast query and 3-element contiguous output runs.

_CACHE = {}


def _build_nc():
    import concourse.bass as bass
    import concourse.mybir as mybir
    from concourse import bacc
    from concourse.tile import TileContext

    f32 = mybir.dt.float32
    nc = bacc.Bacc("TRN2", target_bir_lowering=False, debug=False, num_devices=NCORES)
    key = nc.declare_dram_parameter("key", [NPLANES, KR * W], f32, isOutput=False)
    query = nc.declare_dram_parameter("query", [NPLANES, ROWS * W], f32, isOutput=False)
    out = nc.declare_dram_parameter("out", [NPLANES, ROWS * WO * T], f32, isOutput=True)

    with TileContext(nc) as tc:
        with (
            tc.tile_pool(name="kq", bufs=3) as kq_pool,
            tc.tile_pool(name="op", bufs=4) as out_pool,
        ):
            for g in range(NPLANES // 128):
                for blk in range(ROWS // R):
                    r0 = blk * R
                    ktile = kq_pool.tile([128, (R + 2) * W], f32, tag="key")
                    nc.scalar.dma_start(
                        out=ktile[:],
                        in_=key[g * 128:(g + 1) * 128, r0 * W:(r0 + R + 2) * W],
                    )
                    qtile = kq_pool.tile([128, R * W], f32, tag="query")
                    nc.scalar.dma_start(
                        out=qtile[:],
                        in_=query[g * 128:(g + 1) * 128, r0 * W:(r0 + R) * W],
                    )
                    otile = out_pool.tile([128, R * WO * T], f32, tag="out")
                    kbase = ktile[:]
                    part_pair = list(kbase.ap[0])
                    qb = (
                        qtile[:]
                        .rearrange("p (r w) -> p r w", w=W)[:, :, 1:1 + WO]
                        .unsqueeze(3)
                        .to_broadcast((128, R, WO, K))
                    )
                    ov = otile[:].rearrange(
                        "p (r w kh kw) -> p r w kh kw", w=WO, kh=K, kw=K
                    )
                    for kh in range(K):
                        # overlapping read AP: key[p, r+kh, iw+kw] for
                        # (r, iw, kw) in [R, WO, K]
                        kap = bass.AP(
                            tensor=kbase.tensor,
                            offset=kh * W,
                            ap=[part_pair, [W, R], [1, WO], [1, K]],
                        )
                        nc.vector.tensor_mul(ov[:, :, :, kh, :], kap, qb)
                    nc.sync.dma_start(
                        out=out[g * 128:(g + 1) * 128, r0 * WO * T:(r0 + R) * WO * T],
                        in_=otile[:],
                    )
    nc.compile()
    return nc


def _get_nc():
    if "nc" not in _CACHE:
        _CACHE["nc"] = _build_nc()
    return _CACHE["nc"]


def _make_in_maps(key_map, query_map):
    kflat = np.ascontiguousarray(key_map.reshape(NPLANES, H, W))
    qflat = np.ascontiguousarray(query_map.reshape(NPLANES, H, W))
    in_maps = []
    for i in range(NCORES):
        r0 = ROWS * i
        kshard = np.zeros((NPLANES, KR, W), np.float32)
        nrows = min(KR, H - r0)
        kshard[:, :nrows] = kflat[:, r0:r0 + nrows]
        qshard = np.zeros((NPLANES, ROWS, W), np.float32)
        qrows = min(ROWS, H - (r0 + 1))
        qshard[:, :qrows] = qflat[:, r0 + 1:r0 + 1 + qrows]
        in_maps.append({
            "key": kshard.reshape(NPLANES, KR * W),
            "query": qshard.reshape(NPLANES, ROWS * W),
        })
    return in_maps


def run_spmd(key_map, query_map, trace=False, **kwargs):
    from concourse.bass_utils import run_bass_kernel_spmd

    nc = _get_nc()
    in_maps = _make_in_maps(key_map, query_map)
    res = run_bass_kernel_spmd(
        nc, in_maps, core_ids=list(range(NCORES)), trace=trace, **kwargs
    )
    outs = [res.results[i]["out"].reshape(NPLANES, ROWS, WO, K, K)
            for i in range(NCORES)]
    full = np.concatenate(outs, axis=1)[:, :HO]
    return full.reshape(B, C, HO * WO, K, K), res


def kernel(key_map, query_map, k, stride):
    assert int(k) == K and int(stride) == 1
    key_map = np.asarray(key_map, dtype=np.float32)
    query_map = np.asarray(query_map, dtype=np.float32)
    out, _ = run_spmd(key_map, query_map, trace=False)
    return out
